# revision 16
# baseline (speedup 1.0000x reference)
"""GarNet layer kernel for Trainium2 (8 NeuronCores, data-parallel over batch).

Math (per example b):
    w    = exp(-d_av^2)                      [V=128, S=16]
    hi   = w^T @ fi_v / V                    [S, N=64]
    out  = mean_V(w)[:, None] * hi           [S, N] -> flattened [S*N]

Implementation notes (flipped matmul):
  - Batch B=4096 is sharded 512/core across 8 cores (pure data parallel).
  - Host-side sharding lays inputs out V-major and appends a constant 1.0
    column to fi, so every DMA moves large contiguous runs.
  - Per example ONE fp32 matmul with stationary lhsT = [fi_e | ones]
    [V, 65] and moving rhs = w'_e [V, 16] where w' = exp(-d^2 - ln V).
    Streaming the S=16 side instead of the N=64 side cuts PE rows 4x
    versus the w-stationary orientation.
    Output rows: 0..63 = hi^T, row 64 = sum_V(w)/V = wbar.
  - 64 examples batch into one 2-bank PSUM tile [65, 1024]; one copy
    stages it to SBUF, 8 PE transposes bring it back to [(e,s), 65]
    layout, and one broadcast multiply ps2[...,0:64] * ps2[...,64]
    produces the exact reference quantity (written as bf16, well inside
    the 2e-2 tolerance; the matmul itself stays fp32).
  - DMA issue is spread across SP/Act/Pool queues by a static greedy
    balance that accounts for each engine's compute load; squares and
    epilogue run on DVE, exp on Act.
"""

import numpy as np
from contextlib import ExitStack

import concourse.bass as bass
import concourse.tile as tile
from concourse import mybir
from concourse.bass_utils import run_bass_kernel_spmd

B, V, S, N = 4096, 128, 16, 64
NCORES = 8
BPC = B // NCORES            # examples per core
ECH = 64                     # examples per chunk (PSUM: [65, ECH*S] = 2 banks)
NCH = BPC // ECH
LOG_V = float(np.log(V))
OUT_BF16 = True


def split_multi_waits(nc):
    """The walrus build in this container rejects >1 embedded sem-wait per
    instruction ("Too many sync wait commands" in setupSyncWait). Hoist every
    multi-wait list onto single-wait EventSemaphore instructions immediately
    before the owner on the same engine — identical semantics, since engine
    streams are in order."""
    fn = nc.m.functions[0]
    for block in fn.blocks:
        insts = list(block.instructions)
        changed = False
        new = []
        for inst in insts:
            si = inst.sync_info
            waits = list(si.on_wait) if (si and si.on_wait) else []
            if len(waits) > 1:
                changed = True
                for w in waits:
                    ev = mybir.InstEventSemaphore(
                        name=nc.get_next_instruction_name(), ins=[], outs=[]
                    )
                    ev.engine = inst.engine
                    ev.sync_info = mybir.SyncInfo(on_wait=[w], on_update=[])
                    new.append(ev)
                ups = list(si.on_update) if si.on_update else []
                inst.sync_info = mybir.SyncInfo(on_wait=[], on_update=ups)
            new.append(inst)
        if changed:
            block.instructions = new


def build(bpc=BPC, name="garnet", split_waits=True):
    """Build the per-core Bass module for a shard of `bpc` examples.

    Inputs (host-prepared per core):
      fia   [V, bpc, N+1] f32 — fi transposed V-major, col N = 1.0
      dT    [V, bpc, S]   f32 — d_av transposed V-major
      ident [V, V]        f32 — identity (PE transpose operand)
    Output: out [bpc, S*N] (bf16 on the wire when OUT_BF16, upcast on host).
    """
    assert bpc % ECH == 0
    nchunk = bpc // ECH
    K8 = (ECH * S) // V          # transposes per chunk (8 for ECH=64)

    nc = bass.Bass(name=name)
    f32 = mybir.dt.float32
    odt = mybir.dt.bfloat16 if OUT_BF16 else f32
    fia = nc.dram_tensor("fia", (V, bpc, N + 1), f32, kind="ExternalInput")
    dT = nc.dram_tensor("dT", (V, bpc, S), f32, kind="ExternalInput")
    ident = nc.dram_tensor("ident", (V, V), f32, kind="ExternalInput")
    out = nc.dram_tensor("out", (bpc, S * N), odt, kind="ExternalOutput")

    # Per-chunk DMA engine-costs (ns) in CoreSim's model for greedy balance.
    osz = 2 if OUT_BF16 else 4
    COST_FIA = ECH * (N + 1) * 4 * 0.3855
    COST_DT = ECH * S * 4 * 0.3855
    COST_OUT = max(500.0, K8 * N * osz * 2 * 0.3855)
    ACT_CHUNK = ECH * S * 0.833 + 185.0          # exp
    DVE_CHUNK = (ECH * S * 1.042 + 60.0          # square
                 + K8 * 1.042 + 125.0            # wsb copy
                 + K8 * N * 1.042 + 125.0)       # epilogue mult
    TCOPY_DVE = ECH * S * 1.042 + 125.0
    TCOPY_ACT = ECH * S * 0.833 + 143.0

    with tile.TileContext(nc) as tc, ExitStack() as ctx:
        const = ctx.enter_context(tc.tile_pool(name="const", bufs=1))
        fipool = ctx.enter_context(tc.tile_pool(name="fipool", bufs=6))
        dpool = ctx.enter_context(tc.tile_pool(name="dpool", bufs=8))
        trawpool = ctx.enter_context(tc.tile_pool(name="trawpool", bufs=2))
        ofpool = ctx.enter_context(tc.tile_pool(name="ofpool", bufs=2))
        wspool = ctx.enter_context(tc.tile_pool(name="wspool", bufs=2))
        psp = ctx.enter_context(tc.tile_pool(name="psp", bufs=3, space="PSUM"))
        ps2p = ctx.enter_context(tc.tile_pool(name="ps2p", bufs=2, space="PSUM"))

        id_t = const.tile([V, V], f32)
        bias_t = const.tile([128, 1], f32)
        scr_t = const.tile([128, 1], f32)
        nc.vector.memset(bias_t, -LOG_V)
        # prewarm the Exp activation table so the first real exp is cheap
        nc.scalar.activation(scr_t, bias_t, mybir.ActivationFunctionType.Exp)
        nc.sync.dma_start(out=id_t, in_=ident[:, :])

        issuers = {"sp": nc.sync, "act": nc.scalar, "pool": nc.gpsimd}
        # Pre-charge Act with its total compute (exp) so the greedy only
        # routes DMA there once SP/Pool fill up.
        act_compute = nchunk * ACT_CHUNK + 1283.0
        load = {"sp": 0.0, "act": act_compute, "pool": 0.0}

        def issue(cost, out_ap, in_ap, allow=("sp", "act", "pool")):
            key = min(allow, key=lambda k: load[k])
            load[key] += cost
            issuers[key].dma_start(out=out_ap, in_=in_ap)

        PRE = 4                      # prefetch depth (chunks)
        H = ECH // 2
        fi_tiles, d_tiles = {}, {}

        def issue_loads(c, allow=("sp", "act", "pool")):
            if c >= nchunk:
                return
            b0 = c * ECH
            d_t = dpool.tile([V, ECH, S], f32)
            d_tiles[c] = d_t
            issue(COST_DT, d_t, dT[:, b0 : b0 + ECH, :], ("sp", "pool"))
            fi_t = fipool.tile([V, ECH, N + 1], f32)
            fi_tiles[c] = fi_t
            issue(COST_FIA / 2, fi_t[:, 0:H], fia[:, b0 : b0 + H, :], allow)
            issue(COST_FIA / 2, fi_t[:, H:ECH], fia[:, b0 + H : b0 + ECH, :], allow)

        issue_loads(0, allow=("sp", "pool"))
        for c in range(1, PRE):
            issue_loads(c)

        for c in range(nchunk):
            b0 = c * ECH
            fi_t = fi_tiles.pop(c)
            d_t = d_tiles.pop(c)

            # w' = exp(-(d^2) - lnV), in place
            nc.vector.tensor_mul(d_t, d_t, d_t)
            nc.scalar.activation(d_t, d_t, mybir.ActivationFunctionType.Exp,
                                 scale=-1.0, bias=bias_t)

            # the very last fia load stays off Act so tail exps aren't blocked
            issue_loads(c + PRE,
                        allow=("sp", "act", "pool") if c + PRE < nchunk - 1
                        else ("sp", "pool"))

            p = psp.tile([128, ECH * S], f32)
            for e in range(ECH):
                nc.tensor.matmul(
                    out=p[0 : N + 1, S * e : S * (e + 1)],
                    lhsT=fi_t[:, e, :],
                    rhs=d_t[:, e, :],
                    start=True, stop=True,
                )

            otraw = trawpool.tile([128, ECH * S], f32)
            nc.vector.tensor_copy(otraw[0 : N + 1, :], p[0 : N + 1, :])

            # transpose outputs: 4 blocks of 65 cols per 512-float PSUM bank;
            # one single-bank tile per half so PSUM recycles at finer grain
            HB = K8 // 2
            o_f = ofpool.tile([128, 2, HB, N], odt)
            for h in range(2):
                p2 = ps2p.tile([128, 512], f32)
                for j in range(HB):
                    nc.tensor.transpose(
                        out=p2[:, 65 * j : 65 * j + 65],
                        in_=otraw[0 : N + 1, V * (HB * h + j) : V * (HB * h + j + 1)],
                        identity=id_t[0 : N + 1, 0 : N + 1],
                    )
                p2v = p2[:, 0 : 65 * HB].rearrange("p (j c) -> p j c", j=HB)
                wsb = wspool.tile([128, HB], f32)
                nc.vector.tensor_copy(wsb, p2v[:, :, N])
                nc.vector.tensor_mul(
                    o_f[:, h], p2v[:, :, 0:N],
                    wsb[:, :, None].broadcast_to((128, HB, N)),
                )

            dst = out[b0 : b0 + ECH].rearrange(
                "(h j el) (s n) -> (el s) h j n", h=2, j=HB, s=S
            )
            issue(COST_OUT, dst, o_f, allow=("sp", "pool"))

    if split_waits:
        split_multi_waits(nc)
    return nc


_NC_CACHE = {}


def _get_nc():
    if "nc" not in _NC_CACHE:
        _NC_CACHE["nc"] = build()
    return _NC_CACHE["nc"]


def _prep(fi_v: np.ndarray, d_av: np.ndarray, c: int):
    """Host-side shard + layout for core c."""
    lo, hi = c * BPC, (c + 1) * BPC
    fia = np.empty((V, BPC, N + 1), dtype=np.float32)
    fia[:, :, 0:N] = fi_v[lo:hi].transpose(1, 0, 2)
    fia[:, :, N] = 1.0
    dT = np.ascontiguousarray(d_av[lo:hi].transpose(1, 0, 2))
    return fia, dT


def kernel(fi_v: np.ndarray, d_av: np.ndarray) -> np.ndarray:
    fi_v = np.ascontiguousarray(np.asarray(fi_v, dtype=np.float32))
    d_av = np.ascontiguousarray(np.asarray(d_av, dtype=np.float32))
    assert fi_v.shape == (B, V, S * 4) and d_av.shape == (B, V, S)
    nc = _get_nc()
    ident = np.eye(V, dtype=np.float32)
    in_maps = []
    for c in range(NCORES):
        fia, dT = _prep(fi_v, d_av, c)
        in_maps.append({"fia": fia, "dT": dT, "ident": ident})
    res = run_bass_kernel_spmd(nc, in_maps, core_ids=list(range(NCORES)))
    outs = [np.asarray(res.results[c]["out"]).astype(np.float32)
            for c in range(NCORES)]
    return np.concatenate(outs, axis=0)
